# revision 92
# baseline (speedup 1.0000x reference)
"""Trainium2 Bass kernel for a dense transformer block (B=2, T=2048, C=1024, H=16).

Sharding: DP2 (batch -> core groups {0-3},{4-7}) x TP4 within a group:
  - attention: Megatron head-parallel (4 heads/core). After softmax-normalize,
    each core AllToAll's its y^T head-channels so every core holds the FULL
    1024 y-channels for its OWN 512 token rows; the output projection then
    runs fully local (full w_proj) - no ReduceScatter, 4x less wire traffic.
  - MLP: sequence-parallel (each core computes its 512 rows with the FULL
    fc / proj weights). No other collective.

Row ownership: core at group position p owns rows {512j+128p .. 512j+128p+128}
for j in 0..3 (one 128-row strip per chunk, pipelined AllToAll per chunk).

Device layout notes:
  - Activations feeding matmuls are kept transposed [features, tokens]
    ("^T layout") so every matmul contracts over the partition dim.
  - LN affine params folded into the following weights on the host;
    q-scale (1/sqrt(D)) folded into W_q/b_q; v-bias folded into xres.
  - Softmax: scores^T[k,q] tiles; exp on ScalarE (no max subtraction:
    scores are ~N(0,1), safe); denominator via ones-column appended to V
    (row 64 of the PV matmul output); 1/den partition-broadcast via a
    DRAM bounce on the (otherwise idle) GpSimd SWDGE queue.
  - Matmul operands are fp16 (full PE rate); accumulation, softmax stats,
    residuals and LN are fp32.
  - Engine split: PE matmuls; ScalarE exp/gelu + feed-phase PSUM->SBUF
    copies + qk bias; VectorE LN stats/apply + y normalize; GpSimd
    denominator bounce + collective triggers.
  - MLP strip 3 (gated by the last AllToAll) computed in natural layout
    (free-512 matmuls + rank-1 bias MMs) then PE-transposed into h2gT;
    gpass streams wmp once with a k-outer loop over all 8 PSUM banks.
"""

import os
import sys

import numpy as np

for _p in ("/opt/trn_rl_repo", "/root/.axon_site/_ro/trn_rl_repo"):
    if os.path.isdir(_p) and _p not in sys.path:
        sys.path.insert(0, _p)

import concourse.bass as bass
import concourse.tile as tile
from concourse import bacc, mybir
from concourse.bass_utils import run_bass_kernel_spmd

B, T, C, H = 2, 2048, 1024, 16
D = C // H  # 64
EPS = 1e-5
N_CORES = 8
TP = 4            # tensor-parallel group size
HPC = 4           # heads per core
ROWS = T // TP    # 512 token rows owned per core
F32 = mybir.dt.float32
F16 = mybir.dt.float16  # matmul operand dtype

TT = T // 128     # 16 token tiles
CB = C // 128     # 8 channel blocks
QC = T // 512     # 4 query chunks / row blocks
RG = [[0, 1, 2, 3], [4, 5, 6, 7]]
RG8 = [[0, 1, 2, 3, 4, 5, 6, 7]]

GELU_NAME = "Gelu_apprx_tanh"  # sim_check overrides (sim lacks Gelu)


def _bc(ap, p):
    """Broadcast a DRAM AP across p partitions (prepend stride-0 dim)."""
    return bass.AP(tensor=ap.tensor, offset=ap.offset, ap=[[0, p], *ap.ap])


def build_program():
    nc = bacc.Bacc(
        "TRN2", target_bir_lowering=False, debug=False, num_devices=N_CORES
    )

    # ---- I/O ----
    x_d = nc.dram_tensor("x", [T, C], F16, kind="ExternalInput").ap()
    wqk_d = nc.dram_tensor("wqk", [C, 512], F16, kind="ExternalInput").ap()
    bqk_d = nc.dram_tensor("bqk", [512], F32, kind="ExternalInput").ap()
    wv_d = nc.dram_tensor("wv", [C, 256], F16, kind="ExternalInput").ap()
    wpj_d = nc.dram_tensor("wpj", [C, C], F16, kind="ExternalInput").ap()
    wfc_d = nc.dram_tensor("wfc", [C, 4 * C], F16, kind="ExternalInput").ap()
    bfc_d = nc.dram_tensor("bfc", [4 * C], F32, kind="ExternalInput").ap()
    wmp_d = nc.dram_tensor("wmp", [4 * C, C], F16, kind="ExternalInput").ap()
    bmpn_d = nc.dram_tensor("bmpn", [C], F16, kind="ExternalInput").ap()
    ident_d = nc.dram_tensor("ident", [128, 128], F16, kind="ExternalInput").ap()
    gm_d = nc.dram_tensor("gm", [128, 128], F16, kind="ExternalInput").ap()
    xres_d = nc.dram_tensor("xres", [ROWS, C], F32, kind="ExternalInput").ap()
    gsel_d = nc.dram_tensor("gsel", [1], mybir.dt.int32,
                            kind="ExternalInput").ap()
    out_d = nc.dram_tensor("out", [ROWS, C], F32, kind="ExternalOutput").ap()

    with tile.TileContext(nc) as tc:
        _body(nc, tc, locals())
    nc.compile()
    return nc


def _body(nc, tc, io):
    x_d = io["x_d"]; wqk_d = io["wqk_d"]; bqk_d = io["bqk_d"]; wv_d = io["wv_d"]
    wpj_d = io["wpj_d"]; wfc_d = io["wfc_d"]
    bfc_d = io["bfc_d"]; wmp_d = io["wmp_d"]
    bmpn_d = io["bmpn_d"]
    ident_d = io["ident_d"]; gm_d = io["gm_d"]; xres_d = io["xres_d"]
    gsel_d = io["gsel_d"]; out_d = io["out_d"]

    AF = mybir.ActivationFunctionType
    OP = mybir.AluOpType

    consts = tc.alloc_tile_pool(name="consts", bufs=1)
    dram = tc.alloc_tile_pool(name="dram", bufs=1, space="DRAM")
    ps = tc.alloc_tile_pool(name="ps", bufs=6, space="PSUM")
    ps_av = tc.alloc_tile_pool(name="ps_av", bufs=2, space="PSUM")

    # ---------- constants (DMAs issued lazily below; tiles just declared) ----
    ident = consts.tile([128, 128], F16)
    gm = consts.tile([128, 128], F16)  # -30 above the causal diagonal
    epsb = consts.tile([128, 1], F32)
    bqk_sb = consts.tile([128, 4], F32)
    bfc_sb = consts.tile([128, 32], F32)
    bmp_nat = consts.tile([1, C], F16)
    ones_r1 = consts.tile([1, 128], F16)   # rank-1 bias stationary
    ones_r64 = consts.tile([65, 64], F16)  # row 64: rank-1 bcast stationary
    gs_sb = consts.tile([1, 1], mybir.dt.int32)  # DP-group index (0 or 1)

    # DRAM scratch
    # AllToAll runs over all 8 cores (mesh needs >4); slots for the other
    # DP group carry stale data the receivers ignore. Chunks are shipped in
    # PAIRS ({0,1}, {2,3}) so each slot holds 256 rows per channel -> 512B
    # DMA runs instead of 256B (4x fewer descriptors on the staging DMAs).
    a2a_in = [dram.tile([2 * C, 256], F16, tag=f"a2i{j}", name=f"a2i{j}")
              for j in range(2)]    # [8 dest, 256 ch, 2x128 rows]
    a2a_out = [dram.tile([2 * C, 256], F16, tag=f"a2o{j}", name=f"a2o{j}")
               for j in range(2)]   # [8 src, 256 ch, my 2x128 rows]
    dnrm = [dram.tile([HPC, 512], F16, tag=f"dn{j}", name=f"dn{j}")
            for j in range(QC)]
    # tiny warm-up AllToAll: the first collective after the NRT barrier pays
    # ~11us of CC spin-up; burn it during feed(0) instead of on A2A(01).
    a2a_wu = [dram.tile([8, 16], F16, tag=f"awu{j}", name=f"awu{j}")
              for j in range(2)]

    # ======== Pools (alloc order must honor LIFO release points) ========
    pEG = tc.alloc_tile_pool(name="pEG", bufs=1)   # x_mid (residual base)
    pEF = tc.alloc_tile_pool(name="pEF", bufs=1)   # h_ln^T
    stp2 = tc.alloc_tile_pool(name="stp2", bufs=4)
    xcp = tc.alloc_tile_pool(name="xcp", bufs=2)
    wfcp = tc.alloc_tile_pool(name="wfcp", bufs=16)
    wpjp = tc.alloc_tile_pool(name="wpjp", bufs=1)  # full out-proj weights
    ytp = tc.alloc_tile_pool(name="ytp", bufs=2)    # gathered y^T per chunk
    # ---- attention-lifetime pools (all released before the MLP) ----
    pBC = tc.alloc_tile_pool(name="pBC", bufs=1)   # Q^T/K^T + V natural
    pCD = tc.alloc_tile_pool(name="pCD", bufs=1)   # y^T staging
    dsbp = tc.alloc_tile_pool(name="dsbp", bufs=4)
    ystg = tc.alloc_tile_pool(name="ystg", bufs=2)
    probs = tc.alloc_tile_pool(name="probs", bufs=8)
    pAB = tc.alloc_tile_pool(name="pAB", bufs=1)   # x_ln^T + qkv weights
    xpool = tc.alloc_tile_pool(name="xpool", bufs=3)
    stp = tc.alloc_tile_pool(name="stp", bufs=4)

    xlnT = pAB.tile([128, CB, T], F16, name="xlnT")
    wqk_sb = [pAB.tile([128, 512], F16, tag=f"wqk{i}", name=f"wqk{i}")
              for i in range(CB)]
    wv_sb = [pAB.tile([128, 256], F16, tag=f"wv{i}", name=f"wv{i}")
             for i in range(CB)]
    # Q^T per head, zero-padded to 128 rows (head's 64-row band at its
    # position in the K-pair tile; the other band is zero). Full-partition
    # streaming keeps the PE_HAM activity monitor at full clock during
    # scores (64-contract matmuls otherwise read as "idle" -> K=4/8).
    qz = [pBC.tile([128, T], F16, tag=f"qz{i}", name=f"qz{i}")
          for i in range(4)]
    kT = [pBC.tile([128, T], F16, tag=f"kT{i}", name=f"kT{i}")
          for i in range(2)]  # K^T, 2 heads stacked per tile
    vnat = [pBC.tile([128, 260], F16, tag=f"vnat{i}", name=f"vnat{i}")
            for i in range(TT)]  # 4x(64 V + 1 ones-col)
    yT = [pCD.tile([128, T], F16, tag=f"yT{i}", name=f"yT{i}")
          for i in range(2)]  # y^T, 2 heads per tile
    wpj_sb = [wpjp.tile([128, C], F16, tag=f"wpj{i}", name=f"wpj{i}")
              for i in range(CB)]
    x_mid = [pEG.tile([128, C], F32, tag=f"xmid{i}", name=f"xmid{i}")
             for i in range(QC)]
    hlnT = pEF.tile([128, CB, ROWS], F16, name="hlnT")

    def load_x_chunk(tcn):
        """Prefetch the 4 x tiles of a token chunk (fp16, 256KB each)."""
        xts = []
        for tt in range(4 * tcn, 4 * tcn + 4):
            xt = xpool.tile([128, C], F16, tag="xt", bufs=4)
            nc.sync.dma_start(out=xt, in_=x_d[tt * 128:(tt + 1) * 128, :])
            xts.append(xt)
        return xts

    def feed_ln(tcn, xts):
        """LN1 chain for a chunk (vector/scalar only; x pre-fetched).
        Issued BEFORE the previous chunk's attention so the vector queue
        runs it ahead of that chunk's softmax-normalize work."""
        for i4 in range(4):
            xt = xts[i4]
            st = stp.tile([128, 2, 6], F32, tag="st")
            xr = xt.rearrange("p (g f) -> p g f", g=2)
            nc.vector.bn_stats(out=st[:, 0, :], in_=xr[:, 0, :])
            nc.vector.bn_stats(out=st[:, 1, :], in_=xr[:, 1, :])
            mv = stp.tile([128, 2], F32, tag="mv")
            nc.vector.bn_aggr(out=mv, in_=st)
            rstd = stp.tile([128, 1], F32, tag="rstd")
            nc.scalar.activation(out=rstd, in_=mv[:, 1:2], func=AF.Sqrt,
                                 bias=epsb, scale=1.0)
            nc.vector.reciprocal(out=rstd, in_=rstd)
            # LN applied in place (saves 6KB/partition of SBUF)
            nc.vector.tensor_scalar(out=xt, in0=xt, scalar1=mv[:, 0:1],
                                    scalar2=rstd, op0=OP.subtract,
                                    op1=OP.mult)

    def feed_tiles(tcn, xts, lo, hi):
        """Transpose + V-natural for tiles [lo,hi) of a chunk."""
        for i4 in range(lo, hi):
            tt = 4 * tcn + i4
            xc = xts[i4]
            for cq in range(2):  # two psum banks of 4 transposes each
                pt = ps.tile([128, 512], F16, tag="mm", name="pt")
                for i in range(4):
                    cb = cq * 4 + i
                    nc.tensor.matmul(
                        pt[:, 128 * i:128 * (i + 1)],
                        xc[:, cb * 128:(cb + 1) * 128], ident,
                        is_transpose=True, start=(i == 0), stop=(i == 3))
                nc.scalar.copy(
                    out=xlnT[:, cq * 4:cq * 4 + 4, tt * 128:(tt + 1) * 128],
                    in_=pt.rearrange("p (i f) -> p i f", f=128))
            # V natural for this token tile
            pv = ps.tile([128, 256], F32, tag="mm", name="pv")
            for k in range(CB):
                nc.tensor.matmul(
                    pv, xlnT[:, k, tt * 128:(tt + 1) * 128],
                    wv_sb[k], start=(k == 0), stop=(k == CB - 1))
            nc.gpsimd.memset(
                vnat[tt][:, 0:260].rearrange(
                    "p (h x) -> p h x", x=65)[:, :, 64:65], 1.0)
            nc.scalar.copy(
                out=vnat[tt][:, 0:260].rearrange(
                    "p (h x) -> p h x", x=65)[:, :, 0:64],
                in_=pv.rearrange("p (h x) -> p h x", x=64))
    def feed_qk(tcn):
        """Q^T/K^T columns for a token chunk (bias add on VectorE: it is the
        feed->attention hinge and the scalar queue is busy with copies)."""
        cs = slice(tcn * 512, (tcn + 1) * 512)
        for mt in range(4):
            pq = ps.tile([128, 512], F32, tag="mm", name="pq")
            for k in range(CB):
                nc.tensor.matmul(
                    pq, wqk_sb[k][:, mt * 128:(mt + 1) * 128],
                    xlnT[:, k, tcn * 512:(tcn + 1) * 512],
                    start=(k == 0), stop=(k == CB - 1))
            if mt < 2:  # Q: split the head pair into the padded per-head tiles
                nc.scalar.activation(
                    out=qz[2 * mt][0:64, cs], in_=pq[0:64, :],
                    func=AF.Identity, bias=bqk_sb[0:64, mt:mt + 1], scale=1.0)
                nc.scalar.activation(
                    out=qz[2 * mt + 1][64:128, cs], in_=pq[64:128, :],
                    func=AF.Identity, bias=bqk_sb[64:128, mt:mt + 1],
                    scale=1.0)
            else:
                nc.scalar.activation(
                    out=kT[mt - 2][:, cs], in_=pq,
                    func=AF.Identity, bias=bqk_sb[:, mt:mt + 1], scale=1.0)

    def feed(tcn, xts):
        feed_tiles(tcn, xts, 0, 4)
        feed_qk(tcn)

    GRP = 4  # scores emitted in shape-uniform groups; PV trails one group

    def attention(qc, carry, pieces=()):
        """In-chunk deferred normalize: flush head h-2 at head h's start.
        pieces[i] (if given) is emitted after head i -- independent PE work
        interleaved so the ScalarE exp backlog never stalls the trailing
        PV matmuls for long."""
        for h in range(HPC):
            if 0 < h <= len(pieces):
                pieces[h - 1]()
            off = 64 * (h % 2)
            qh = qz[h]   # 128 rows: head band + zeros
            kh = kT[h // 2]  # other head's rows hit Q's zero band
            nkb = 4 * qc + 4
            py = ps_av.tile([128, 512], F32, tag="py", name="py")
            pend = []
            for g0 in range(0, nkb, GRP):
                prs = []
                for kb in range(g0, min(g0 + GRP, nkb)):
                    j = kb - 4 * qc
                    lo = max(j, 0) * 128  # fully-masked columns skipped
                    pss = ps.tile([128, 512], F32, tag="mm", name="pss")
                    nc.tensor.matmul(
                        pss[:, lo:512], kh[:, kb * 128:(kb + 1) * 128],
                        qh[:, qc * 512 + lo:(qc + 1) * 512],
                        start=True, stop=(j < 0))
                    if j >= 0:  # causal diagonal: accumulate -30 above it
                        nc.tensor.matmul(
                            pss[:, lo:lo + 128], gm, ident,
                            start=False, stop=True, skip_group_check=True)
                    pr = probs.tile([128, 512], F16, tag="pr")
                    nc.scalar.activation(out=pr[:, lo:512],
                                         in_=pss[:, lo:512], func=AF.Exp)
                    prs.append((kb, lo, pr))
                if g0 == 0 and carry:
                    # one deferred y^T normalization per head (spacing)
                    _flush_one(carry.pop(0))
                for pkb, plo, ppr in pend:  # PV for the previous group
                    nc.tensor.matmul(
                        py[0:65, plo:512], vnat[pkb][:, h * 65:h * 65 + 65],
                        ppr[:, plo:512], start=(pkb == 0),
                        stop=(pkb == nkb - 1))
                pend = prs
            for pkb, plo, ppr in pend:
                nc.tensor.matmul(
                    py[0:65, plo:512], vnat[pkb][:, h * 65:h * 65 + 65],
                    ppr[:, plo:512], start=(pkb == 0), stop=(pkb == nkb - 1))
            # 1/denominator: reciprocal straight from the PV PSUM bank, then
            # partition-broadcast via DRAM bounce on GpSimd's SWDGE queue
            # (keeps the Sync DMA queue + ScalarE free during attention).
            # The last head of the last chunk gates the final AllToAll:
            # broadcast via a rank-1 PE matmul instead (~2.5us vs ~5us).
            d16 = dsbp.tile([65, 512], F16, tag="d16", bufs=1)
            with nc.allow_low_precision(reason="fp16 1/den (~5e-4 on y)"):
                nc.vector.reciprocal(out=d16[64:65, :], in_=py[64:65, :])
            if qc == QC - 1 and h == HPC - 1:
                rbc = None
            else:
                nc.gpsimd.dma_start(out=dnrm[qc][h, :], in_=d16[64:65, :])
                rbc = dsbp.tile([64, 512], F16, tag="rbc", bufs=2)
                nc.gpsimd.dma_start(out=rbc, in_=_bc(dnrm[qc][h, :], 64))
            if rbc is None:
                rbc = (d16,)
            # y^T columns stored dest-major within the chunk pair
            # (col = pr*1024 + d*256 + c*128 + r) so the AllToAll staging
            # DMA gets 512B-contiguous runs.
            pr, cno = divmod(qc, 2)
            ysl = yT[h // 2][off:off + 64,
                             pr * 1024:(pr + 1) * 1024].rearrange(
                "p (d c r) -> p c d r", d=TP, c=2)[:, cno]
            carry.append((h, ysl, rbc, py))
        return carry

    def _flush_one(ent):
        # y^T = py * (1/den): single TT op reading the PV PSUM bank
        h0, ysl0, rbc0, py0 = ent
        if isinstance(rbc0, tuple):
            d16x = rbc0[0]
            rb = ps.tile([64, 512], F32, tag="mm", name="rbps")
            nc.tensor.matmul(rb, ones_r64[64:65, :], d16x[64:65, :],
                             start=True, stop=True)
            rbs = ystg.tile([64, 512], F16, tag="rbs", bufs=1)
            nc.scalar.copy(out=rbs, in_=rb)
            yst = ystg.tile([64, 512], F16, tag="yst", bufs=1)
            nc.vector.tensor_mul(yst, py0[0:64, :], rbs)
            nc.sync.dma_start(out=ysl0,
                              in_=yst.rearrange("p (d r) -> p d r", d=TP))
            return
        if h0 % 2 == 0:
            nc.vector.tensor_mul(
                ysl0, py0[0:64, :].rearrange("p (d r) -> p d r", d=TP),
                rbc0.rearrange("p (d r) -> p d r", d=TP))
        else:
            yst = ystg.tile([64, 512], F16, tag="yst", bufs=1)
            nc.vector.tensor_mul(yst, py0[0:64, :], rbc0)
            nc.sync.dma_start(out=ysl0,
                              in_=yst.rearrange("p (d r) -> p d r", d=TP))

    def flush_norm(carry):
        for ent in carry:
            _flush_one(ent)
        carry.clear()

    def ydma_half(pr, h2):
        """Ship one y^T half (2 heads) of chunk pair pr to the A2A buffer."""
        tgt = a2a_in[pr]
        in_ap = yT[h2][:, pr * 1024:(pr + 1) * 1024].rearrange(
            "p (d f) -> p d f", d=TP)
        out_v = tgt.rearrange("(e d h p) f -> e h p d f", e=2, d=TP, h=2)
        nc.sync.dma_start(out=out_v[0][h2], in_=in_ap)
        nc.gpsimd.dma_start(out=out_v[1][h2], in_=in_ap)

    def a2a_go(pr):
        nc.gpsimd.collective_compute(
            "AllToAll", mybir.AluOpType.bypass, replica_groups=RG8,
            ins=[a2a_in[pr].opt()], outs=[a2a_out[pr].opt()])

    _gcache = []

    def gsel():
        """DP-group index (0/1) as a runtime scalar, loaded from the gsel
        input (partition_id() is unsupported by this runtime)."""
        if not _gcache:
            reg = nc.sync.alloc_register("greg")
            nc.sync.reg_load(reg, gs_sb[0:1, 0:1])
            _gcache.append(nc.sync.snap(reg, donate=True,
                                        min_val=0, max_val=1))
        return _gcache[0]

    ytl_pair = {}

    def load_ytl(pr):
        """Load the gathered y^T of chunk pair pr (my group's half)."""
        ytl = ytp.tile([128, CB, 256], F16, tag="ytl", bufs=1,
                       name=f"ytl{pr}")
        src = a2a_out[pr].rearrange("(g2 k p) r -> g2 p k r",
                                    g2=2, p=128)[gsel()]
        for cno in range(2):
            nc.sync.dma_start(out=ytl[:, :, 128 * cno:128 * (cno + 1)],
                              in_=src[:, :, 128 * cno:128 * (cno + 1)])
        ytl_pair[pr] = ytl

    def post_proj(qc):
        """Gathered-y slice -> local out-projection -> residual + LN2 stats."""
        ytl = ytl_pair[qc // 2]
        c0 = (qc % 2) * 128
        xo = xcp.tile([128, C], F32, tag="xo", bufs=1)
        nc.sync.dma_start(out=xo, in_=xres_d[qc * 128:(qc + 1) * 128, :])
        for cc in range(2):
            pp = ps.tile([128, 512], F32, tag="mm", name=f"pp{qc}_{cc}")
            for k in range(CB):
                nc.tensor.matmul(
                    pp, ytl[:, k, c0:c0 + 128],
                    wpj_sb[k][:, cc * 512:(cc + 1) * 512],
                    start=(k == 0), stop=(k == CB - 1))
            nc.vector.tensor_add(
                x_mid[qc][:, cc * 512:(cc + 1) * 512], pp,
                xo[:, cc * 512:(cc + 1) * 512])
        st = stp2.tile([128, 2, 6], F32, tag="st2", bufs=2)
        xr = x_mid[qc].rearrange("p (g f) -> p g f", g=2)
        nc.vector.bn_stats(out=st[:, 0, :], in_=xr[:, 0, :])
        nc.vector.bn_stats(out=st[:, 1, :], in_=xr[:, 1, :])
        mv = stp2.tile([128, 2], F32, tag="mv2", bufs=2)
        nc.vector.bn_aggr(out=mv, in_=st)
        rstd = stp2.tile([128, 1], F32, tag="rstd2", bufs=2)
        nc.scalar.activation(out=rstd, in_=mv[:, 1:2], func=AF.Sqrt,
                             bias=epsb, scale=1.0)
        nc.vector.reciprocal(out=rstd, in_=rstd)
        return mv, rstd

    def post_lnT(qc, mr):
        """LN2 apply + transpose into the hlnT strip."""
        mv, rstd = mr
        xc = xcp.tile([128, C], F16, tag="xc2", bufs=1)
        nc.vector.tensor_scalar(out=xc, in0=x_mid[qc], scalar1=mv[:, 0:1],
                                scalar2=rstd, op0=OP.subtract, op1=OP.mult)
        for cq in range(2):
            pt = ps.tile([128, 512], F16, tag="mm", name="pt2")
            for i in range(4):
                cb = cq * 4 + i
                nc.tensor.matmul(
                    pt[:, 128 * i:128 * (i + 1)],
                    xc[:, cb * 128:(cb + 1) * 128], ident,
                    is_transpose=True, start=(i == 0), stop=(i == 3))
            nc.scalar.copy(
                out=hlnT[:, cq * 4:cq * 4 + 4, qc * 128:(qc + 1) * 128],
                in_=pt.rearrange("p (i f) -> p i f", f=128))

    def post(qc):
        post_lnT(qc, post_proj(qc))

    def load_wfc_group(mg, pool, t_load):
        # one 256KB load every ~3us: never saturates the HW DMA queues, so
        # small critical DMAs always slip through
        wg = []
        for k in range(CB):
            with tc.tile_wait_until(t_load + 0.003 * k):
                w = pool.tile([128, 1024], F16, tag="wfc",
                              name=f"wfc_{mg}_{k}")
                nc.gpsimd.dma_start(
                    out=w, in_=wfc_d[k * 128:(k + 1) * 128,
                                     mg * 1024:(mg + 1) * 1024])
                wg.append(w)
        return wg

    def fc_mg(mg, t0, t1, wg, h2gT):
        # h2^T = gelu(wfc^T @ h_ln^T + b_fc), one mg weight group, rows [t0,t1)
        n0, n1 = t0 * 128, t1 * 128
        for mt in range(8):
            m = mg * 8 + mt
            pf = ps.tile([128, 512], F32, tag="mm", name="pf")
            for k in range(CB):
                nc.tensor.matmul(
                    pf[:, 0:n1 - n0], wg[k][:, mt * 128:(mt + 1) * 128],
                    hlnT[:, k, n0:n1], start=(k == 0),
                    stop=(k == CB - 1))
            nc.scalar.activation(
                out=h2gT[:, m, n0:n1], in_=pf[:, 0:n1 - n0],
                func=getattr(AF, GELU_NAME),
                bias=bfc_sb[:, m:m + 1], scale=1.0)

    def g_pass(h2gT, psg, wmpp, outp):
        # out rows = h2g^T.T @ wmp + bmp + x_mid; k-outer loop, 8 PSUM banks
        pg = [psg.tile([128, 512], F32, tag="pg", name=f"pg{i}")
              for i in range(8)]
        for i in range(8):
            cc = i % 2
            nc.tensor.matmul(pg[i], ones_r1,
                             bmp_nat[:, cc * 512:(cc + 1) * 512],
                             start=True, stop=False)
        for k in range(32):
            with tc.tile_wait_until(0.230 + 0.002 * k):
                wm = wmpp.tile([128, 1024], F16, tag="wmp")
                nc.sync.dma_start(
                    out=wm, in_=wmp_d[k * 128:(k + 1) * 128, :])
            for t in range(4):
                for cc in range(2):
                    nc.tensor.matmul(
                        pg[t * 2 + cc], h2gT[:, k, t * 128:(t + 1) * 128],
                        wm[:, cc * 512:(cc + 1) * 512],
                        start=False, stop=(k == 31))
        xof = xcp.tile([128, C], F32, tag="xo", bufs=1)  # reuse xres buf
        for t in range(4):
            for cc in range(2):
                i = t * 2 + cc
                if i % 3 == 0:
                    ot = outp.tile([128, 512], F32, tag="ot")
                else:
                    ot = xof[:, 512 * (i % 3 - 1):512 * (i % 3)]
                nc.vector.tensor_add(ot, pg[i],
                                     x_mid[t][:, cc * 512:(cc + 1) * 512])
                nc.sync.dma_start(
                    out=out_d[t * 128:(t + 1) * 128,
                              cc * 512:(cc + 1) * 512],
                    in_=ot)

    # ====== unified software pipeline over token/query chunks ======
    # DMA issue order = priority: x chunk 0 first, then ident (transposes),
    # wv/wqk (feed 0), gm (attn 0); heavier / later-needed loads follow.
    xts0 = load_x_chunk(0)
    nc.sync.dma_start(out=ident, in_=ident_d)
    nc.sync.dma_start(out=gs_sb, in_=gsel_d.rearrange("(p f) -> p f", p=1))
    nc.vector.memset(epsb, EPS)
    nc.vector.memset(ones_r1, 1.0)
    nc.vector.memset(ones_r64[64:65, :], 1.0)
    for h in range(4):  # zero the unused 64-row band of each padded Q^T
        zoff = 0 if h % 2 else 64
        nc.gpsimd.memset(qz[h][zoff:zoff + 64, :], 0.0)
    for k in range(CB):
        nc.sync.dma_start(out=wv_sb[k], in_=wv_d[k * 128:(k + 1) * 128, :])
    for k in range(CB):
        nc.sync.dma_start(out=wqk_sb[k], in_=wqk_d[k * 128:(k + 1) * 128, :])
    nc.sync.dma_start(out=bqk_sb, in_=bqk_d.rearrange("(m p) -> p m", p=128))
    nc.sync.dma_start(out=gm, in_=gm_d)
    xts1 = load_x_chunk(1)
    for k in range(CB):  # full out-projection weights (2MB, spread out)
        with tc.tile_wait_until(0.020 + 0.004 * k):
            nc.sync.dma_start(out=wpj_sb[k],
                              in_=wpj_d[k * 128:(k + 1) * 128, :])

    carry = []
    nc.gpsimd.collective_compute(
        "AllToAll", mybir.AluOpType.bypass, replica_groups=RG8,
        ins=[a2a_wu[0].opt()], outs=[a2a_wu[1].opt()])
    # warm the Sqrt activation table while the x tiles stream in, so
    # feed_ln(0)'s first rstd does not pay the table load on the critical
    # path (attention's Exp reload is unavoidable either way).
    nc.scalar.activation(out=epsb, in_=epsb, func=AF.Sqrt)
    nc.vector.memset(epsb, EPS)
    feed_ln(0, xts0)
    feed(0, xts0)
    feed_ln(1, xts1)
    attention(0, carry, pieces=(
        lambda: feed_tiles(1, xts1, 0, 2),
        lambda: feed_tiles(1, xts1, 2, 4),
        lambda: feed_qk(1)))
    flush_norm(carry)
    xts2 = load_x_chunk(2)
    # deferred const loads (needed from post / fc onwards)
    nc.sync.dma_start(out=bmp_nat, in_=_bc(bmpn_d, 1))
    nc.sync.dma_start(out=bfc_sb, in_=bfc_d.rearrange("(m p) -> p m", p=128))
    feed_ln(2, xts2)
    attention(1, carry, pieces=(
        lambda: feed_tiles(2, xts2, 0, 2),
        lambda: feed_tiles(2, xts2, 2, 4),
        lambda: feed_qk(2)))
    ydma_half(0, 0)
    flush_norm(carry)
    ydma_half(0, 1)
    a2a_go(0)
    xts3 = load_x_chunk(3)
    feed_ln(3, xts3)
    attention(2, carry, pieces=(
        lambda: feed_tiles(3, xts3, 0, 2),
        lambda: feed_tiles(3, xts3, 2, 4),
        lambda: feed_qk(3)))
    flush_norm(carry)
    load_ytl(0)
    mrs = {}
    attention(3, carry, pieces=(
        lambda: mrs.update(m0=post_proj(0)),
        lambda: mrs.update(m1=post_proj(1)),
        lambda: (post_lnT(0, mrs["m0"]), post_lnT(1, mrs["m1"]),
                 ydma_half(1, 0))))
    # release feed/attention pools before the flush chain so the engine
    # drains overlap it instead of serializing after
    stp.release()
    xpool.release()
    pAB.release()
    probs.release()
    flush_norm(carry)
    ydma_half(1, 1)
    a2a_go(1)
    # wfc groups 0/1 streamed during attention (sync queue, spread)
    wgs = {mg: load_wfc_group(mg, wfcp, 0.050 + 0.030 * mg)
           for mg in range(2)}
    # MLP pools open only after the attention pools close (SBUF budget)
    pFG = tc.alloc_tile_pool(name="pFG", bufs=1)   # gelu(h2)^T
    wfcp2 = tc.alloc_tile_pool(name="wfcp2", bufs=16)  # wfc groups 2/3
    wmpp = tc.alloc_tile_pool(name="wmpp", bufs=4)
    outp = tc.alloc_tile_pool(name="outp", bufs=1)
    h2gT = pFG.tile([128, 32, ROWS], F16, name="h2gT")
    for mg in range(2, 4):
        wgs[mg] = load_wfc_group(mg, wfcp2, 0.0)
    load_ytl(1)
    # fc ordered so groups 0/1 (resident since attention) run first, covering
    # the pair-{2,3} AllToAll + post(2)/post(3) + the group-2/3 streams.
    fc_mg(0, 0, 2, wgs[0], h2gT)
    fc_mg(1, 0, 2, wgs[1], h2gT)
    fc_mg(2, 0, 2, wgs[2], h2gT)
    fc_mg(3, 0, 2, wgs[3], h2gT)
    mr2 = post_proj(2)
    mr3 = post_proj(3)
    post_lnT(2, mr2)
    post_lnT(3, mr3)
    fc_mg(0, 2, 4, wgs[0], h2gT)
    fc_mg(1, 2, 4, wgs[1], h2gT)
    fc_mg(2, 2, 4, wgs[2], h2gT)
    fc_mg(3, 2, 4, wgs[3], h2gT)
    # re-pool PSUM: one bank per (strip, cc) output for the k-outer g-pass
    ps_av.release()
    ps.release()
    psg = tc.alloc_tile_pool(name="psg", bufs=8, space="PSUM")
    g_pass(h2gT, psg, wmpp, outp)

    psg.release()
    outp.release()
    wmpp.release()
    wfcp2.release()
    pFG.release()
    ystg.release()
    dsbp.release()
    pCD.release()
    pBC.release()
    ytp.release()
    wpjp.release()
    wfcp.release()
    xcp.release()
    stp2.release()
    pEF.release()
    pEG.release()
    dram.release()
    consts.release()


_CACHED = None


def _get_program():
    global _CACHED
    if _CACHED is None:
        _CACHED = build_program()
    return _CACHED


def _prep_inputs(inputs):
    """Fold LN params into weights and build the 8 per-core input maps."""
    x = np.asarray(inputs["x"], np.float32)
    ln1_w = np.asarray(inputs["ln1_w"], np.float32)
    ln1_b = np.asarray(inputs["ln1_b"], np.float32)
    w_attn = np.asarray(inputs["w_attn"], np.float32)
    b_attn = np.asarray(inputs["b_attn"], np.float32)
    w_proj = np.asarray(inputs["w_proj"], np.float32)
    b_proj = np.asarray(inputs["b_proj"], np.float32)
    ln2_w = np.asarray(inputs["ln2_w"], np.float32)
    ln2_b = np.asarray(inputs["ln2_b"], np.float32)
    w_fc = np.asarray(inputs["w_fc"], np.float32)
    b_fc = np.asarray(inputs["b_fc"], np.float32)
    w_mp = np.asarray(inputs["w_mlp_proj"], np.float32)
    b_mp = np.asarray(inputs["b_mlp_proj"], np.float32)

    Wa = ln1_w[:, None] * w_attn                      # [C, 3C]
    Ba = b_attn + ln1_b @ w_attn                      # [3C]
    s = 1.0 / np.sqrt(D)
    Wq = Wa[:, 0:C] * s
    Bq = Ba[0:C] * s
    Wk = Wa[:, C:2 * C]
    Bk = Ba[C:2 * C]
    Wv = Wa[:, 2 * C:3 * C]
    Bv = Ba[2 * C:3 * C]
    bproj_eff = (b_proj + Bv @ w_proj).astype(np.float32)

    Wfc = (ln2_w[:, None] * w_fc).astype(np.float32)
    Bfc = (b_fc + ln2_b @ w_fc).astype(np.float32)

    ident = np.eye(128, dtype=np.float16)
    gm = np.where(np.arange(128)[:, None] < np.arange(128)[None, :],
                  np.float16(-30.0), np.float16(0.0))

    in_maps = []
    for c in range(N_CORES):
        g, p = divmod(c, TP)
        hs = slice(HPC * D * p, HPC * D * (p + 1))    # 256 cols/rows per core
        wqk = np.ascontiguousarray(
            np.concatenate([Wq[:, hs], Wk[:, hs]], axis=1), np.float16)
        bqk = np.ascontiguousarray(
            np.concatenate([Bq[hs], Bk[hs]]), np.float32)
        xres = np.concatenate(
            [x[g][512 * j + 128 * p:512 * j + 128 * p + 128]
             for j in range(QC)], axis=0) + bproj_eff[None, :]
        in_maps.append({
            "x": np.ascontiguousarray(x[g]).astype(np.float16),
            "xres": np.ascontiguousarray(xres),
            "wqk": wqk,
            "bqk": bqk,
            "wv": np.ascontiguousarray(Wv[:, hs]).astype(np.float16),
            "wpj": w_proj.astype(np.float16),
            "wfc": Wfc.astype(np.float16),
            "bfc": Bfc,
            "wmp": w_mp.astype(np.float16),
            "bmpn": b_mp.astype(np.float16),
            "ident": ident,
            "gm": gm,
            "gsel": np.array([g], np.int32),
        })
    return in_maps


def _gather(results):
    out = np.empty((B, T, C), np.float32)
    for c in range(N_CORES):
        g, p = divmod(c, TP)
        for j in range(QC):
            out[g, 512 * j + 128 * p:512 * j + 128 * p + 128, :] = \
                results[c]["out"][128 * j:128 * (j + 1)]
    return out


def kernel(**inputs) -> np.ndarray:
    nc = _get_program()
    in_maps = _prep_inputs(inputs)
    res = run_bass_kernel_spmd(nc, in_maps, list(range(N_CORES)))
    return _gather(res.results)


if __name__ == "__main__":
    print("building program...")
    _get_program()
    print("built ok")
